# revision 4
# baseline (speedup 1.0000x reference)
"""Two-layer GAT (heads=1) + global mean pool on 8 Trainium2 NeuronCores.

Sharding: nodes dst-partitioned into 8 contiguous blocks; each core owns the
edges whose dst lands in its block (plus self-loops). Per layer:
  1. producer (f32 PE): h_ext = feats @ [W | W@a_s | W@a_d] for the local
     6250-node shard -> bf16 gather table row [h(128) | 1.0 | 0pad*3 | a_s f32]
     (134 bf16 cols = 268B rows); alpha_d kept in a local f32 DRAM array.
  2. AllGather the table (8 x 6272 rows -> 50176).
  3. edge phase: per 128-node group, gather the h-rows of all incident edges
     with one multi-offset indirect DMA (uniform grid of CPG 128-edge chunks,
     pad edges have dstloc=-1), gather alpha_s / alpha_d (4B indirect),
     e = leakyrelu(a_s+a_d), ex = exp(e) (no max-shift; scores are O(10) so
     f32 exp is safe), build S[e,n] = (iota==dstloc)*ex in one fused DVE
     tensor_scalar (bf16), and accumulate psum[n, 0:130] += S.T @ [h | 1 | 0]
     over the CPG chunks: col 128 is the softmax denominator.
  4. epilogue: feats = psum[:, :128] / (s + 1e-16) (+ bias on layer 2).
Pooling: per-group one-hot(batch) f32 matmul accumulated over layer-2 tiles,
AllReduce, scale by 1/count.
"""
import sys
import numpy as np
import ml_dtypes

for _p in ("/opt/trn_rl_repo", "/root/.axon_site/_ro/trn_rl_repo"):
    if _p not in sys.path:
        sys.path.append(_p)

import concourse.bass as bass
import concourse.tile as tile
from concourse import mybir
from concourse.bass_utils import run_bass_kernel_spmd

BF16 = ml_dtypes.bfloat16
F32 = mybir.dt.float32
BF = mybir.dt.bfloat16
I32 = mybir.dt.int32

NEG = 0.2
TW = 132          # table row (bf16): h(0:128) | one(128) | pad(129) | as f32 (130:132) = 264B
GW = 130          # matmul rhs view per edge: h(128) + one + pad
ASF = 65          # f32 column of alpha_s in the bitcast row (byte 260)


class Cfg:
    def __init__(self, n, p, nloc, npad, cpg, num_graphs):
        self.N, self.P, self.NLOC, self.NPAD, self.CPG = n, p, nloc, npad, cpg
        self.NG = npad // 128
        self.NUM_GRAPHS = num_graphs


# ----------------------------------------------------------------- host prep

def host_prep(cfg, src, dst, x, batch, W1, as1, ad1, b1, W2, as2, ad2, b2):
    """Returns (in_maps list per core, shared consts dict)."""
    P, NLOC, NPAD, NG = cfg.P, cfg.NLOC, cfg.NPAD, cfg.NG
    core = dst // NLOC
    percore = []
    cpg = cfg.CPG
    for p in range(P):
        m = core == p
        s = src[m].astype(np.int64)
        dl = (dst[m] - p * NLOC).astype(np.int64)
        order = np.argsort(dl, kind="stable")
        s, dl = s[order], dl[order]
        grp = dl >> 7
        cnt = np.bincount(grp, minlength=NG)
        assert int(np.ceil(cnt / 128).max()) <= cpg, "CPG too small"
        # flat slot of edge k (within its group): c*128 + i
        start = np.zeros(NG, np.int64)
        start[1:] = np.cumsum(cnt)[:-1]
        pos = np.arange(len(dl)) - start[grp]
        slot = grp * (cpg * 128) + pos
        M = NG * cpg * 128
        f_src = np.zeros(M, np.int64)          # pad edges gather row 0
        f_dl = np.full(M, -1.0, np.float32)    # pad -> S column none
        f_dr = np.full(M, ((NPAD - 1) % 128) * NG + ((NPAD - 1) >> 7), np.int64)
        q = s // NLOC
        f_src[slot] = q * NPAD + (s - q * NLOC)
        f_dl[slot] = (dl & 127).astype(np.float32)
        f_dr[slot] = ((dl & 127) * NG + (dl >> 7))
        # layout [128, NG*CPG]: edge (g,c,i) at [i, g*CPG+c]
        def lay(a, dt):
            return np.ascontiguousarray(
                a.reshape(NG, cpg, 128).transpose(2, 0, 1).reshape(128, NG * cpg)
            ).astype(dt)
        srcidx = lay(f_src, np.int32)
        dstloc = lay(f_dl, np.float32)
        dstrow = lay(f_dr, np.int32)

        xT = np.zeros((128, NPAD), np.float32)
        xT[:, :NLOC] = x[p * NLOC:(p + 1) * NLOC].T
        bl = batch[p * NLOC:(p + 1) * NLOC].astype(np.int64)
        oh = np.zeros((128, NG * cfg.NUM_GRAPHS), np.float32)
        n = np.arange(NLOC)
        oh[n & 127, (n >> 7) * cfg.NUM_GRAPHS + bl] = 1.0
        percore.append(dict(xT=xT, srcidx=srcidx, dstloc=dstloc, dstrow=dstrow,
                            onehot=oh))

    def wext(W, a_s, a_d):
        w = np.concatenate([W, (W @ a_s)[:, None], (W @ a_d)[:, None]], axis=1)
        return np.ascontiguousarray(w).astype(np.float32)  # [128, 130]

    counts = np.bincount(batch.astype(np.int64), minlength=cfg.NUM_GRAPHS)
    shared = dict(
        w1ext=wext(W1, as1, ad1),
        w2ext=wext(W2, as2, ad2),
        b1col=np.ascontiguousarray(b1[:, None]).astype(np.float32),
        b2tile=np.ascontiguousarray(np.tile(b2[None, :], (128, 1))).astype(np.float32),
        iota=np.ascontiguousarray(
            np.tile(np.arange(128, dtype=np.float32)[None, :], (128, 1))).astype(BF16),
        rcounts=(1.0 / np.maximum(counts, 1.0))[:, None].astype(np.float32),
    )
    in_maps = [{**pc, **shared} for pc in percore]
    return in_maps


# ------------------------------------------------------------- bass program

def _split_waits(bir: bytes, limit: int = 1) -> bytes:
    """This container's walrus rejects >1 sync-wait condition per
    instruction (setupSyncWait 'Too many sync wait commands'). Split the
    surplus waits onto preceding same-engine NoOps — identical semantics
    (all waits still complete before the instruction issues)."""
    import json
    d = json.loads(bir)
    for f in d["functions"]:
        for b in f["blocks"]:
            out = []
            for ins in b["instructions"]:
                si = ins.get("sync_info") or {}
                ow = si.get("on_wait") or []
                k = 0
                while len(ow) > limit:
                    head, ow = ow[:limit], ow[limit:]
                    out.append({
                        "name": f"{ins['name']}-sw{k}",
                        "opcode": "NoOp",
                        "engine": ins["engine"],
                        "ins": [], "outs": [],
                        "sync_info": {"on_update": [], "on_wait": head},
                        "debug": ins.get("debug", 0),
                    })
                    k += 1
                si["on_wait"] = ow
                ins["sync_info"] = si
                out.append(ins)
            b["instructions"] = out
    return json.dumps(d).encode()


def build_program(cfg):
    P, NPAD, NG, CPG = cfg.P, cfg.NPAD, cfg.NG, cfg.CPG
    NGR = cfg.NUM_GRAPHS
    nc = bass.Bass()

    xT_d = nc.dram_tensor("xT", [128, NPAD], F32, kind="ExternalInput")
    srcidx_d = nc.dram_tensor("srcidx", [128, NG * CPG], I32, kind="ExternalInput")
    dstloc_d = nc.dram_tensor("dstloc", [128, NG * CPG], F32, kind="ExternalInput")
    dstrow_d = nc.dram_tensor("dstrow", [128, NG * CPG], I32, kind="ExternalInput")
    onehot_d = nc.dram_tensor("onehot", [128, NG * NGR], F32, kind="ExternalInput")
    w1ext_d = nc.dram_tensor("w1ext", [128, 130], F32, kind="ExternalInput")
    w2ext_d = nc.dram_tensor("w2ext", [128, 130], F32, kind="ExternalInput")
    b1col_d = nc.dram_tensor("b1col", [128, 1], F32, kind="ExternalInput")
    b2tile_d = nc.dram_tensor("b2tile", [128, 128], F32, kind="ExternalInput")
    iota_d = nc.dram_tensor("iota", [128, 128], BF, kind="ExternalInput")
    rcounts_d = nc.dram_tensor("rcounts", [NGR, 1], F32, kind="ExternalInput")

    feats_out = nc.dram_tensor("feats", [NPAD, 128], F32, kind="ExternalOutput")
    pooled_out = nc.dram_tensor("pooled", [NGR, 128], F32, kind="ExternalOutput")

    replica = [list(range(P))]

    with tile.TileContext(nc) as tc:
        with tc.tile_pool(name="const", bufs=1) as cp, \
             tc.tile_pool(name="dram", bufs=1, space="DRAM") as dp, \
             tc.tile_pool(name="work", bufs=2) as wp, \
             tc.tile_pool(name="sgen", bufs=4) as sp, \
             tc.tile_pool(name="small", bufs=4) as mp, \
             tc.tile_pool(name="ppool", bufs=2, space="PSUM") as pp, \
             tc.tile_pool(name="pprod", bufs=2, space="PSUM") as pq, \
             tc.tile_pool(name="ppole", bufs=1, space="PSUM") as pg:

            # ---------------- constants to SBUF ----------------
            xT_sb = cp.tile([128, NPAD], F32)
            nc.sync.dma_start(xT_sb[:], xT_d[:])
            srcidx_sb = cp.tile([128, NG * CPG], I32)
            nc.sync.dma_start(srcidx_sb[:], srcidx_d[:])
            dstloc_sb = cp.tile([128, NG * CPG], F32)
            nc.sync.dma_start(dstloc_sb[:], dstloc_d[:])
            dstrow_sb = cp.tile([128, NG * CPG], I32)
            nc.sync.dma_start(dstrow_sb[:], dstrow_d[:])
            onehot_sb = cp.tile([128, NG * NGR], F32)
            nc.sync.dma_start(onehot_sb[:], onehot_d[:])
            w1ext_sb = cp.tile([128, 130], F32)
            nc.sync.dma_start(w1ext_sb[:], w1ext_d[:])
            w2ext_sb = cp.tile([128, 130], F32)
            nc.sync.dma_start(w2ext_sb[:], w2ext_d[:])
            b1col_sb = cp.tile([128, 1], F32)
            nc.sync.dma_start(b1col_sb[:], b1col_d[:])
            b2tile_sb = cp.tile([128, 128], F32)
            nc.sync.dma_start(b2tile_sb[:], b2tile_d[:])
            iota_sb = cp.tile([128, 128], BF)
            nc.sync.dma_start(iota_sb[:], iota_d[:])
            rcounts_sb = cp.tile([NGR, 1], F32)
            nc.sync.dma_start(rcounts_sb[:], rcounts_d[:])
            from concourse.masks import make_identity
            ident_sb = cp.tile([128, 128], F32)
            make_identity(nc, ident_sb[:])
            feats1_sb = cp.tile([128, NG * 128], F32)

            # ---------------- internal DRAM ----------------
            tab1_loc = dp.tile([NPAD, TW], BF)
            tab1_full = dp.tile([P * NPAD, TW], BF, addr_space="Shared")
            tab2_loc = dp.tile([NPAD, TW], BF)
            tab2_full = dp.tile([P * NPAD, TW], BF, addr_space="Shared")
            ad1_dram = dp.tile([NPAD, 1], F32)
            ad2_dram = dp.tile([NPAD, 1], F32)
            pool_in = dp.tile([NGR, 128], F32)
            pool_red = dp.tile([NGR, 128], F32, addr_space="Shared")

            # ---------------- producers ----------------
            def assemble(g, ps, tab_loc, ad_sb):
                tt = wp.tile([128, TW], BF, tag="tabt", bufs=3)
                nc.vector.tensor_copy(tt[:, 0:128], ps[:, 0:128])
                nc.gpsimd.memset(tt[:, 128:129], 1.0)
                nc.gpsimd.memset(tt[:, 129:130], 0.0)
                nc.vector.tensor_copy(tt[:, 130:132].bitcast(F32), ps[:, 128:129])
                nc.vector.tensor_copy(ad_sb[:, g:g + 1], ps[:, 129:130])
                nc.sync.dma_start(tab_loc[g * 128:(g + 1) * 128, :], tt[:])

            def produce(layer):
                ad_sb = wp.tile([128, NG], F32, tag=f"ad{layer}", bufs=1)
                for g in range(NG):
                    ps = pq.tile([128, 130], F32, tag="prodps")
                    if layer == 1:
                        nc.tensor.matmul(ps[:], lhsT=xT_sb[:, g * 128:(g + 1) * 128],
                                         rhs=w1ext_sb[:], start=True, stop=True)
                    else:
                        pt = pq.tile([128, 128], F32, tag="prodtp")
                        nc.tensor.transpose(pt[:], feats1_sb[:, g * 128:(g + 1) * 128],
                                            ident_sb[:])
                        ft = wp.tile([128, 128], F32, tag="f1t", bufs=3)
                        nc.vector.tensor_scalar(
                            out=ft[:], in0=pt[:], scalar1=b1col_sb[:, 0:1],
                            scalar2=None, op0=mybir.AluOpType.add)
                        nc.tensor.matmul(ps[:], lhsT=ft[:], rhs=w2ext_sb[:],
                                         start=True, stop=True)
                    assemble(g, ps, tab1_loc if layer == 1 else tab2_loc, ad_sb)
                ad_dram = ad1_dram if layer == 1 else ad2_dram
                nc.sync.dma_start(
                    ad_dram[:, 0].rearrange("(p g) -> p g", p=128), ad_sb[:])

            # ---------------- edge phase ----------------
            def edge_phase(layer, tab_full, ad_dram):
                for g in range(NG):
                    gt = wp.tile([128, CPG * TW], BF, tag="gt", bufs=3)
                    adg = mp.tile([128, CPG], F32, tag="adg")
                    for c in range(CPG):
                        gc = g * CPG + c
                        nc.gpsimd.indirect_dma_start(
                            out=gt[:, c * TW:(c + 1) * TW], out_offset=None,
                            in_=tab_full[:, :],
                            in_offset=bass.IndirectOffsetOnAxis(
                                ap=srcidx_sb[:, gc:gc + 1], axis=0))
                        nc.gpsimd.indirect_dma_start(
                            out=adg[:, c:c + 1], out_offset=None,
                            in_=ad_dram[:, :],
                            in_offset=bass.IndirectOffsetOnAxis(
                                ap=dstrow_sb[:, gc:gc + 1], axis=0))
                    # alpha_s rides the row gather: f32 col ASF of each row
                    asg = gt[:].bitcast(F32)[:, ASF::TW // 2]
                    e_t = mp.tile([128, CPG], F32, tag="e_t")
                    nc.vector.tensor_add(e_t[:], asg, adg[:])
                    lk = mp.tile([128, CPG], F32, tag="lk")
                    nc.vector.tensor_scalar_mul(lk[:], e_t[:], NEG)
                    mx = mp.tile([128, CPG], F32, tag="mx")
                    nc.vector.tensor_tensor(
                        out=mx[:], in0=e_t[:], in1=lk[:], op=mybir.AluOpType.max)
                    ex_t = mp.tile([128, CPG], F32, tag="ex_t")
                    nc.scalar.activation(ex_t[:], mx[:],
                                         mybir.ActivationFunctionType.Exp)
                    ps = pp.tile([128, GW], F32, tag="edgeps")
                    for c in range(CPG):
                        s_bf = sp.tile([128, 128], BF, tag="sgen")
                        nc.vector.tensor_scalar(
                            out=s_bf[:], in0=iota_sb[:],
                            scalar1=dstloc_sb[:, g * CPG + c:g * CPG + c + 1],
                            scalar2=ex_t[:, c:c + 1],
                            op0=mybir.AluOpType.is_equal,
                            op1=mybir.AluOpType.mult)
                        nc.tensor.matmul(ps[:], lhsT=s_bf[:],
                                         rhs=gt[:, c * TW:c * TW + GW],
                                         start=(c == 0), stop=(c == CPG - 1))
                    ssum = mp.tile([128, 1], F32, tag="ssum")
                    nc.vector.tensor_scalar_add(ssum[:], ps[:, 128:129], 1e-16)
                    rec = mp.tile([128, 1], F32, tag="rec")
                    nc.vector.reciprocal(rec[:], ssum[:])
                    if layer == 1:
                        nc.vector.tensor_scalar_mul(
                            feats1_sb[:, g * 128:(g + 1) * 128], ps[:, 0:128],
                            rec[:, 0:1])
                    else:
                        f2 = wp.tile([128, 128], F32, tag="f2", bufs=3)
                        nc.vector.tensor_scalar_mul(f2[:], ps[:, 0:128], rec[:, 0:1])
                        f2b = wp.tile([128, 128], F32, tag="f2b", bufs=3)
                        nc.vector.tensor_add(f2b[:], f2[:], b2tile_sb[:])
                        nc.sync.dma_start(feats_out[g * 128:(g + 1) * 128, :], f2b[:])
                        nc.tensor.matmul(
                            pool_ps[:], lhsT=onehot_sb[:, g * NGR:(g + 1) * NGR],
                            rhs=f2b[:], start=(g == 0), stop=(g == NG - 1),
                            skip_group_check=True)

            produce(1)
            nc.gpsimd.collective_compute(
                "AllGather", mybir.AluOpType.bypass, replica_groups=replica,
                ins=[tab1_loc.opt()], outs=[tab1_full.opt()])
            edge_phase(1, tab1_full, ad1_dram)
            produce(2)
            nc.gpsimd.collective_compute(
                "AllGather", mybir.AluOpType.bypass, replica_groups=replica,
                ins=[tab2_loc.opt()], outs=[tab2_full.opt()])
            pool_ps = pg.tile([NGR, 128], F32)
            edge_phase(2, tab2_full, ad2_dram)

            # ---------------- pooled ----------------
            pl = wp.tile([NGR, 128], F32, tag="pl", bufs=1)
            nc.vector.tensor_copy(pl[:], pool_ps[:])
            nc.sync.dma_start(pool_in[:], pl[:])
            nc.gpsimd.collective_compute(
                "AllReduce", mybir.AluOpType.add, replica_groups=replica,
                ins=[pool_in.opt()], outs=[pool_red.opt()])
            plr = wp.tile([NGR, 128], F32, tag="plr", bufs=1)
            nc.sync.dma_start(plr[:], pool_red[:])
            plo = wp.tile([NGR, 128], F32, tag="plo", bufs=1)
            nc.vector.tensor_scalar_mul(plo[:], plr[:], rcounts_sb[:, 0:1])
            nc.sync.dma_start(pooled_out[:], plo[:])

    _orig_to_json = nc.to_json_bytes
    nc.to_json_bytes = lambda *a, **kw: _split_waits(_orig_to_json(*a, **kw))
    return nc


# ------------------------------------------------------------------ kernel

def compute_cpg(cfg, src, dst):
    core = dst // cfg.NLOC
    cpg = 1
    for p in range(cfg.P):
        dl = dst[core == p] - p * cfg.NLOC
        cnt = np.bincount(dl >> 7, minlength=cfg.NG)
        cpg = max(cpg, int(np.ceil(cnt.max() / 128)))
    return cpg


def run(cfg, inputs, trace=False):
    x = np.asarray(inputs["x"], np.float32)
    ei = np.asarray(inputs["edge_index"]).astype(np.int64)
    batch = np.asarray(inputs["batch"]).astype(np.int64)
    args = [np.asarray(inputs[k], np.float32)
            for k in ("W1", "as1", "ad1", "b1", "W2", "as2", "ad2", "b2")]
    loops = np.arange(cfg.N, dtype=np.int64)
    src = np.concatenate([ei[0], loops])
    dst = np.concatenate([ei[1], loops])
    cfg.CPG = compute_cpg(cfg, src, dst)
    in_maps = host_prep(cfg, src, dst, x, batch, *args)
    nc = build_program(cfg)
    res = run_bass_kernel_spmd(nc, in_maps, list(range(cfg.P)), trace=trace)
    feats = np.concatenate(
        [res.results[p]["feats"][:cfg.NLOC] for p in range(cfg.P)], axis=0)
    pooled = res.results[0]["pooled"]
    return (feats.astype(np.float32), pooled.astype(np.float32)), res


def kernel(**inputs):
    cfg = Cfg(n=50000, p=8, nloc=6250, npad=6272, cpg=19, num_graphs=64)
    (feats, pooled), _ = run(cfg, inputs)
    return feats, pooled
